# revision 12
# baseline (speedup 1.0000x reference)
"""Distributed multi-head attention kernel for 8 TRN2 NeuronCores.

Head-parallel tensor parallelism: each core owns 2 of the 16 heads.
Compute in bf16 (f32 PSUM accumulation). Scores are computed transposed
(ST[j,i] = k_j . q_i) so that the softmax denominator rides the PV matmul
via a ones-column in V and no transpose of P is needed.
No max-subtraction softmax: logits are O(10), exp stays in f32 range.

v3 structure:
  - Attention runs over 8 groups of 512 query tokens; each (group, jt)
    iteration computes BOTH heads' [128,512] score blocks into one
    [128,1024] PSUM tile so softmax needs ONE exp ACTIVATE and ONE
    bf16 multiply per iteration (ACT is the pace-setting engine).
  - PSUM: 2x st [128,1024] (4 banks) + 2x ot [65,512] (2 banks) +
    2x fill [128,512] (2 banks) = 8 banks.
  - PE filler work (batch-1 QKV, output projection) is interleaved into
    the attention iterations so the tensor engine stream stays dense and
    the HAM clock gate stays at full rate.
  - Token-parallel output projection: per 1024-token chunk, an AllToAll
    exchanges head-blocks for token-blocks (8x less wire than AllGather),
    then each core projects its own 128-token slice against the full
    w_proj, writing out.T ([tokens, D]) directly.
"""

import os
import numpy as np
import ml_dtypes

import concourse.bass as bass
import concourse.mybir as mybir
import concourse.tile as tile
from concourse import bacc
from concourse.bass_utils import run_bass_kernel_spmd
from concourse.masks import make_identity

BF16 = mybir.dt.bfloat16
F32 = mybir.dt.float32
AF = mybir.ActivationFunctionType
OP = mybir.AluOpType

NCORES = 8
B, N, D, H, HD = 2, 2048, 1024, 16, 64
NT = B * N
HPC = H // NCORES     # 2 heads per core
NB = N // 512         # 4 groups of 512 tokens per batch
MASK_NEG = -30000.0

LAST_EXEC_TIME_NS = None


def _build_graph():
    nc = bacc.Bacc("TRN2", target_bir_lowering=False, debug=False, num_devices=NCORES)

    xT = nc.declare_dram_parameter("xT", [D, NT], BF16, isOutput=False)
    wqkvT = nc.declare_dram_parameter("wqkvT", [D, 6 * HD], BF16, isOutput=False)
    # cb2[b, gg, j, p*512+i] = exp(bias[h_p, j, gg*512+i] + maskval[b, j, ...])
    cb2 = nc.declare_dram_parameter("cb2", [B, NB, N, HPC * 512], BF16,
                                    isOutput=False)
    wp = nc.declare_dram_parameter("wp", [D, D], BF16, isOutput=False)
    bp = nc.declare_dram_parameter("bp", [1, D], F32, isOutput=False)
    # out.T: this core's 4x128 tokens (one 128-block per 1024-token chunk)
    out_ext = nc.declare_dram_parameter("out", [4 * 128, D], F32, isOutput=True)

    # AllToAll bounce buffers: per chunk, 8 shards of [128 rows, 128 tokens]
    cc_in = nc.dram_tensor("cc_in", [4, NCORES, 128, 128], BF16)
    cc_out = nc.dram_tensor("cc_out", [4, NCORES, 128, 128], BF16)
    cc_warm_in = nc.dram_tensor("cc_warm_in", [NCORES, 128, 128], BF16)
    cc_warm_out = nc.dram_tensor("cc_warm_out", [NCORES, 128, 128], BF16)
    groups = [list(range(NCORES))]

    KT = D // 128  # 8 contraction tiles

    with tile.TileContext(nc) as tc:
        with (
            tc.tile_pool(name="persist", bufs=1) as persist,
            tc.tile_pool(name="xt", bufs=40) as xt_pool,
            tc.tile_pool(name="st", bufs=2, space="PSUM") as st_pool,
            tc.tile_pool(name="ot_ps", bufs=2, space="PSUM") as ot_pool,
            tc.tile_pool(name="fill", bufs=2, space="PSUM") as fill_pool,
            tc.tile_pool(name="sw", bufs=6) as sw_pool,
            tc.tile_pool(name="pw", bufs=4) as pw_pool,
            tc.tile_pool(name="cbt", bufs=12) as cb_pool,
            tc.tile_pool(name="small", bufs=4) as small_pool,
            tc.tile_pool(name="og", bufs=16) as og_pool,
            tc.tile_pool(name="outt", bufs=3) as out_pool,
        ):
            # ---------------- warmups ----------------
            nc.sync.dma_start(out=cc_warm_in[0], in_=xT[0:128, 0:128])
            nc.gpsimd.collective_compute(
                "AllToAll", OP.bypass, replica_groups=groups,
                ins=[cc_warm_in[:, :, :].opt()], outs=[cc_warm_out[:, :, :].opt()])

            # ---------------- persistent tensors ----------------
            w_sb = persist.tile([128, KT * 6 * HD], BF16, tag="w")
            for k in range(KT):
                nc.sync.dma_start(
                    out=w_sb[:, k * 6 * HD:(k + 1) * 6 * HD],
                    in_=wqkvT[k * 128:(k + 1) * 128, :])
            wp_sb = persist.tile([128, KT * D], BF16, tag="wp")
            for k in range(KT):
                nc.sync.dma_start(out=wp_sb[:, k * D:(k + 1) * D],
                                  in_=wp[k * 128:(k + 1) * 128, :])
            id_sb = persist.tile([128, 64], BF16, tag="ident")
            make_identity(nc, id_sb[0:64, :])
            make_identity(nc, id_sb[64:128, :])
            ones_sb = persist.tile([1, 64], BF16, tag="ones")
            nc.vector.memset(ones_sb[:], 1.0)

            # exp activation-table preload
            warm_exp = persist.tile([1, 64], BF16, tag="warmexp")
            nc.scalar.activation(warm_exp[:], id_sb[0:1, :], AF.Exp)

            # bias row replicated across partitions via K=1 outer product
            onesf = persist.tile([1, 128], F32, tag="onesf")
            nc.vector.memset(onesf[:], 1.0)
            bp_row = persist.tile([1, D], F32, tag="bprow")
            nc.sync.dma_start(out=bp_row[:], in_=bp[:, :])
            bp_rep = persist.tile([128, D], F32, tag="bprep")
            for h in range(2):
                rp = fill_pool.tile([128, 512], F32, tag="fill", name=f"bpr{h}")
                nc.tensor.matmul(rp[:], lhsT=onesf[:],
                                 rhs=bp_row[:, h * 512:(h + 1) * 512],
                                 start=True, stop=True)
                nc.vector.tensor_copy(bp_rep[:, h * 512:(h + 1) * 512], rp[:])

            # ---------------- QKV machinery ----------------
            # per-batch SBUF tensors (no cross-batch AP aliasing)
            qkv_sb = [[persist.tile([128, N], BF16, tag=f"qkv{b}{m}",
                                    name=f"qkv{b}{m}") for m in range(3)]
                      for b in range(B)]
            vaug = [persist.tile([128, HPC * 16 * 65], BF16, tag=f"vaug{b}",
                                 name=f"vaug{b}") for b in range(B)]
            for b in range(B):
                nc.vector.memset(vaug[b][:], 1.0)

            xt_sb = {}

            def emit_xt(b, nch, eng=None):
                # nch is batch-local (0..3); loads 8 k-tiles of 512 tokens
                eng = eng or nc.sync
                tiles = []
                for k in range(KT):
                    t = xt_pool.tile([128, 512], BF16, tag="xt",
                                     name=f"xt{b}_{nch}_{k}")
                    eng.dma_start(
                        out=t[:],
                        in_=xT[k * 128:(k + 1) * 128,
                               b * N + nch * 512: b * N + (nch + 1) * 512])
                    tiles.append(t)
                xt_sb[(b, nch)] = tiles

            def emit_qkv_half(b, nch, m, half, on_act):
                # one half (4 k-tiles) of a [128,512] qkv chunk accumulation
                key = (b, nch, m)
                if half == 0:
                    acc = fill_pool.tile([128, 512], F32, tag="fill",
                                         name=f"acc{b}_{nch}_{m}")
                    qkv_acc[key] = acc
                acc = qkv_acc[key]
                for k in range(4 * half, 4 * half + 4):
                    nc.tensor.matmul(
                        acc[:],
                        lhsT=w_sb[:, k * 6 * HD + m * 128:
                                  k * 6 * HD + (m + 1) * 128],
                        rhs=xt_sb[(b, nch)][k][:],
                        start=(k == 0), stop=(k == KT - 1))
                if half == 1:
                    nsl = slice(nch * 512, (nch + 1) * 512)
                    if on_act:
                        nc.scalar.copy(qkv_sb[b][m][:, nsl], acc[:])
                    else:
                        nc.vector.tensor_copy(qkv_sb[b][m][:, nsl], acc[:])
                    qkv_acc.pop(key)

            def emit_vt(b, nch):
                # transpose the v chunk into V_aug lhsT tiles
                v_sb = qkv_sb[b][2]
                for jj in range(4):
                    nloc = nch * 512 + jj * 128
                    jt = nloc // 128
                    for p in range(HPC):
                        tp = fill_pool.tile([128, 512], BF16, tag="fill",
                                            name=f"tp{b}_{nch}_{jj}_{p}")
                        nc.tensor.transpose(
                            tp[:, 0:64], v_sb[p * 64:(p + 1) * 64,
                                              nloc:nloc + 128],
                            id_sb[p * 64:(p + 1) * 64, :])
                        base = (p * 16 + jt) * 65
                        nc.vector.tensor_copy(vaug[b][:, base:base + 64],
                                              tp[:, 0:64])

            qkv_acc = {}

            # ---------------- cbt prefetch machinery ----------------
            PF = 10
            cb_tiles = {}

            def emit_cbt(T):
                if T >= 128:
                    return
                Gq, jtq = divmod(T, 16)
                bq, ggq = divmod(Gq, NB)
                t = cb_pool.tile([128, 1024], BF16, tag="cbt", name=f"cbt{T}")
                nc.sync.dma_start(
                    out=t[:], in_=cb2[bq, ggq, jtq * 128:(jtq + 1) * 128, :])
                cb_tiles[T] = t

            # ---------------- phase 0: QKV for batch 0 ----------------
            # order on the sync queue: first 2 b0 x-chunks, then the cbt
            # primes (needed at attention iter 0), then the rest of b0's x.
            # b1's x goes on the scalar HWDGE ring: zero sync contention.
            emit_xt(0, 0)
            emit_xt(0, 1)
            for T in range(PF):
                emit_cbt(T)
            emit_xt(0, 2)
            emit_xt(0, 3)
            for nch in range(4):
                emit_xt(1, nch, eng=nc.scalar)
            for nch in range(4):
                for m in range(3):
                    emit_qkv_half(0, nch, m, 0, True)
                    emit_qkv_half(0, nch, m, 1, True)
                emit_vt(0, nch)

            # batch-1 QKV is interleaved into batch-0 attention: small
            # halves mid-group, dense 8-matmul bursts at group boundaries
            # (a >=3.4us contiguous PE stream re-warms the HAM clock gate)
            qkv_half_seq = [(u // 3, u % 3, h) for u in range(12)
                            for h in (0, 1)]
            half_idx = [0]

            def next_qkv_half():
                if half_idx[0] >= len(qkv_half_seq):
                    return
                nch, m, h = qkv_half_seq[half_idx[0]]
                half_idx[0] += 1
                emit_qkv_half(1, nch, m, h, False)
                if m == 2 and h == 1:
                    emit_vt(1, nch)

            # ---------------- projection machinery ----------------
            og_tiles = {}

            def emit_og(c):
                tiles = []
                for k in range(KT):
                    ogt = og_pool.tile([128, 128], BF16, tag="og",
                                       name=f"og{c}_{k}")
                    nc.gpsimd.dma_start(out=ogt[:], in_=cc_out[c, k])
                    tiles.append(ogt)
                og_tiles[c] = tiles

            def emit_proj_half(c, h):
                # out[t, o-half] for this core's 128-token slice of chunk c
                pps = fill_pool.tile([128, 512], F32, tag="fill",
                                     name=f"pps{c}_{h}")
                for k in range(KT):
                    nc.tensor.matmul(
                        pps[:],
                        lhsT=og_tiles[c][k][:],
                        rhs=wp_sb[:, k * D + h * 512: k * D + (h + 1) * 512],
                        start=(k == 0), stop=(k == KT - 1))
                if h == 1:
                    og_tiles.pop(c)
                outt = out_pool.tile([128, 512], F32, tag="outt",
                                     name=f"outt{c}_{h}")
                nc.vector.tensor_tensor(outt[:], pps[:],
                                        bp_rep[:, h * 512:(h + 1) * 512],
                                        OP.add)
                nc.sync.dma_start(
                    out=out_ext[c * 128:(c + 1) * 128,
                                h * 512:(h + 1) * 512],
                    in_=outt[:])

            # ---------------- attention ----------------
            # oT per 1024-token chunk
            oT_c = [persist.tile([128, 1024], BF16, tag=f"oT{c}",
                                 name=f"oT{c}") for c in range(4)]

            def emit_pv(ots, b, item):
                jt, pw = item
                for p in range(HPC):
                    base = (p * 16 + jt) * 65
                    nc.tensor.matmul(
                        ots[p][:],
                        lhsT=vaug[b][:, base:base + 65],
                        rhs=pw[:, p * 512:(p + 1) * 512],
                        start=(jt == 0), stop=(jt == 15))

            for G in range(2 * NB):          # 8 groups of 512 query tokens
                b, gg = divmod(G, NB)
                c = (b * N + gg * 512) // 1024   # 1024-token chunk index
                isl = slice(gg * 512, (gg + 1) * 512)
                ots = [ot_pool.tile([65, 512], F32, tag="ot",
                                    name=f"ot{G}_{p}") for p in range(HPC)]
                pend = []
                for jt in range(16):
                    T = G * 16 + jt
                    emit_cbt(T + PF)
                    cbt = cb_tiles.pop(T)
                    st = st_pool.tile([128, 1024], F32, tag="st")
                    for p in range(HPC):
                        nc.tensor.matmul(
                            st[:, p * 512:(p + 1) * 512],
                            lhsT=qkv_sb[b][1][p * 64:(p + 1) * 64,
                                              jt * 128:(jt + 1) * 128],
                            rhs=qkv_sb[b][0][p * 64:(p + 1) * 64, isl],
                            start=True, stop=True)
                    raw = sw_pool.tile([128, 1024], BF16, tag="sw")
                    nc.scalar.activation(raw[:], st[:], AF.Exp)
                    pw = pw_pool.tile([128, 1024], BF16, tag="pw")
                    nc.vector.tensor_tensor(pw[:], raw[:], cbt[:], OP.mult)
                    pend.append((jt, pw))
                    if len(pend) > 2:
                        emit_pv(ots, b, pend.pop(0))
                    # batch-1 QKV halves mid-group during batch-0 groups
                    if G < NB and jt in (2, 5, 8, 11):
                        next_qkv_half()
                if G < NB:
                    # boundary warm-burst: one full 8-matmul unit
                    next_qkv_half()
                    next_qkv_half()
                while pend:
                    emit_pv(ots, b, pend.pop(0))
                if G == 4:
                    emit_og(1)
                    emit_proj_half(0, 0)
                    emit_proj_half(0, 1)
                elif G == 5:
                    emit_proj_half(1, 0)
                    emit_proj_half(1, 1)
                # normalize: 1/sum broadcast via K=1 outer product, then
                # scale the head rows into the chunk's oT tile
                for p in range(HPC):
                    ot = ots[p]
                    recf = small_pool.tile([1, 512], F32, tag="recf")
                    sums = small_pool.tile([1, 512], F32, tag="sums")
                    nc.vector.tensor_copy(sums[:], ot[64:65, :])
                    nc.vector.reciprocal_approx_fast(recf[:], sums[:])
                    rep = small_pool.tile([64, 512], F32, tag="rep")
                    nc.gpsimd.partition_broadcast(rep[:], recf[:])
                    nc.vector.tensor_tensor(
                        oT_c[c][p * 64:(p + 1) * 64,
                                (G % 2) * 512:(G % 2) * 512 + 512],
                        ot[0:64, :], rep[:], OP.mult)
                if G % 2 == 1:
                    # chunk complete: 8 token-block shards -> AllToAll
                    for j in range(NCORES):
                        nc.sync.dma_start(
                            out=cc_in[c, j],
                            in_=oT_c[c][:, j * 128:(j + 1) * 128])
                    nc.gpsimd.collective_compute(
                        "AllToAll", OP.bypass, replica_groups=groups,
                        ins=[cc_in[c].opt()], outs=[cc_out[c].opt()])
                    if c == 1:
                        emit_og(0)
                    elif c == 2:
                        emit_og(2)   # waits on A2A(2) on the idle SWDGE q

            # tail
            emit_proj_half(2, 0)
            emit_proj_half(2, 1)
            emit_og(3)
            emit_proj_half(3, 0)
            emit_proj_half(3, 1)

    nc.compile()
    return nc


_GRAPH = None


def _get_graph():
    global _GRAPH
    if _GRAPH is None:
        _GRAPH = _build_graph()
    return _GRAPH


def kernel(x, attn_bias, attn_mask, w_qkv, w_proj, b_proj):
    global LAST_EXEC_TIME_NS
    bf16 = ml_dtypes.bfloat16
    x = np.asarray(x, np.float32)
    attn_bias = np.asarray(attn_bias, np.float32)
    attn_mask = np.asarray(attn_mask)
    w_qkv = np.asarray(w_qkv, np.float32)
    w_proj = np.asarray(w_proj, np.float32)
    b_proj = np.asarray(b_proj, np.float32)

    scale = np.float32(HD ** -0.5)
    xT = np.ascontiguousarray(x.reshape(NT, D).T).astype(bf16)
    wq, wk, wv = w_qkv[0:D], w_qkv[D:2 * D], w_qkv[2 * D:3 * D]
    maskvalT = np.where(attn_mask, np.float32(MASK_NEG),
                        np.float32(0.0)).transpose(0, 2, 1)  # [B, j, i]
    biasT = attn_bias[0].transpose(0, 2, 1)                  # [H, j, i]
    wp_np = np.ascontiguousarray(w_proj.T).astype(bf16)      # [D(d), D(o)]
    bp_np = b_proj.reshape(1, D).astype(np.float32)

    in_maps = []
    for cid in range(NCORES):
        hs = [HPC * cid + p for p in range(HPC)]
        wcols = np.concatenate(
            [wq[h * HD:(h + 1) * HD] * scale for h in hs]
            + [wk[h * HD:(h + 1) * HD] for h in hs]
            + [wv[h * HD:(h + 1) * HD] for h in hs], axis=0)   # [384, D]
        wqkvT_np = np.ascontiguousarray(wcols.T).astype(bf16)  # [D, 384]
        cb_np = np.empty((B, NB, N, HPC * 512), dtype=bf16)
        for b in range(B):
            for p, h in enumerate(hs):
                with np.errstate(under="ignore"):
                    plane = np.exp(biasT[h] + maskvalT[b]).astype(bf16)
                for gg in range(NB):
                    cb_np[b, gg, :, p * 512:(p + 1) * 512] = \
                        plane[:, gg * 512:(gg + 1) * 512]
        in_maps.append({"xT": xT, "wqkvT": wqkvT_np, "cb2": cb_np,
                        "wp": wp_np, "bp": bp_np})

    nc = _get_graph()
    trace = bool(os.environ.get("BASS_PROF"))
    res = run_bass_kernel_spmd(nc, in_maps, core_ids=list(range(NCORES)),
                               trace=trace)
    LAST_EXEC_TIME_NS = res.exec_time_ns
    # res[c]["out"]: [4*128, D] = out.T rows for tokens c*... reassemble:
    # global token (chunk ch, core cid, i) = ch*1024 + cid*128 + i
    outT = np.empty((NT, D), np.float32)
    for cid in range(NCORES):
        o = res.results[cid]["out"]
        for ch in range(4):
            outT[ch * 1024 + cid * 128: ch * 1024 + (cid + 1) * 128] = \
                o[ch * 128:(ch + 1) * 128]
    return outT.reshape(B, N, D).astype(np.float32)


# revision 14
# speedup vs baseline: 1.0322x; 1.0322x over previous
"""Distributed multi-head attention kernel for 8 TRN2 NeuronCores.

Head-parallel tensor parallelism: each core owns 2 of the 16 heads.
Compute in bf16 (f32 PSUM accumulation). Scores are computed transposed
(ST[j,i] = k_j . q_i) so that the softmax denominator rides the PV matmul
via a ones-column in V and no transpose of P is needed.
No max-subtraction softmax: logits are O(10), exp stays in f32 range.

v3 structure:
  - Attention runs over 8 groups of 512 query tokens; each (group, jt)
    iteration computes BOTH heads' [128,512] score blocks into one
    [128,1024] PSUM tile so softmax needs ONE exp ACTIVATE and ONE
    bf16 multiply per iteration (ACT is the pace-setting engine).
  - PSUM: 2x st [128,1024] (4 banks) + 2x ot [65,512] (2 banks) +
    2x fill [128,512] (2 banks) = 8 banks.
  - PE filler work (batch-1 QKV, output projection) is interleaved into
    the attention iterations so the tensor engine stream stays dense and
    the HAM clock gate stays at full rate.
  - Token-parallel output projection: per 1024-token chunk, an AllToAll
    exchanges head-blocks for token-blocks (8x less wire than AllGather),
    then each core projects its own 128-token slice against the full
    w_proj, writing out.T ([tokens, D]) directly.
"""

import os
import numpy as np
import ml_dtypes

import concourse.bass as bass
import concourse.mybir as mybir
import concourse.tile as tile
from concourse import bacc
from concourse.bass_utils import run_bass_kernel_spmd
from concourse.masks import make_identity

BF16 = mybir.dt.bfloat16
F32 = mybir.dt.float32
AF = mybir.ActivationFunctionType
OP = mybir.AluOpType

NCORES = 8
B, N, D, H, HD = 2, 2048, 1024, 16, 64
NT = B * N
HPC = H // NCORES     # 2 heads per core
NB = N // 512         # 4 groups of 512 tokens per batch
MASK_NEG = -30000.0

LAST_EXEC_TIME_NS = None


def _build_graph():
    nc = bacc.Bacc("TRN2", target_bir_lowering=False, debug=False, num_devices=NCORES)

    xT = nc.declare_dram_parameter("xT", [D, NT], BF16, isOutput=False)
    wqkvT = nc.declare_dram_parameter("wqkvT", [D, 6 * HD], BF16, isOutput=False)
    # cb2[b, gg, j, p*512+i] = exp(bias[h_p, j, gg*512+i] + maskval[b, j, ...])
    cb2 = nc.declare_dram_parameter("cb2", [B, NB, N, HPC * 512], BF16,
                                    isOutput=False)
    wp = nc.declare_dram_parameter("wp", [D, D], BF16, isOutput=False)
    bp = nc.declare_dram_parameter("bp", [1, D], F32, isOutput=False)
    # out.T: this core's 4x128 tokens (one 128-block per 1024-token chunk)
    out_ext = nc.declare_dram_parameter("out", [4 * 128, D], F32, isOutput=True)

    # AllToAll bounce buffers: per chunk, 8 shards of [128 rows, 128 tokens]
    cc_in = nc.dram_tensor("cc_in", [4, NCORES, 128, 128], BF16)
    cc_out = nc.dram_tensor("cc_out", [4, NCORES, 128, 128], BF16)
    cc_warm_in = nc.dram_tensor("cc_warm_in", [NCORES, 128, 128], BF16)
    cc_warm_out = nc.dram_tensor("cc_warm_out", [NCORES, 128, 128], BF16)
    groups = [list(range(NCORES))]

    KT = D // 128  # 8 contraction tiles

    with tile.TileContext(nc) as tc:
        with (
            tc.tile_pool(name="persist", bufs=1) as persist,
            tc.tile_pool(name="xt", bufs=64) as xt_pool,
            tc.tile_pool(name="st", bufs=2, space="PSUM") as st_pool,
            tc.tile_pool(name="ot_ps", bufs=2, space="PSUM") as ot_pool,
            tc.tile_pool(name="fill", bufs=2, space="PSUM") as fill_pool,
            tc.tile_pool(name="sw", bufs=6) as sw_pool,
            tc.tile_pool(name="pw", bufs=4) as pw_pool,
            tc.tile_pool(name="cbt", bufs=10) as cb_pool,
            tc.tile_pool(name="small", bufs=4) as small_pool,
            tc.tile_pool(name="og", bufs=16) as og_pool,
            tc.tile_pool(name="outt", bufs=2) as out_pool,
        ):
            # ---------------- warmups ----------------
            nc.sync.dma_start(out=cc_warm_in[0], in_=xT[0:128, 0:128])
            nc.gpsimd.collective_compute(
                "AllToAll", OP.bypass, replica_groups=groups,
                ins=[cc_warm_in[:, :, :].opt()], outs=[cc_warm_out[:, :, :].opt()])

            # ---------------- persistent tensors ----------------
            w_sb = persist.tile([128, KT * 6 * HD], BF16, tag="w")
            for k in range(KT):
                nc.sync.dma_start(
                    out=w_sb[:, k * 6 * HD:(k + 1) * 6 * HD],
                    in_=wqkvT[k * 128:(k + 1) * 128, :])
            wp_sb = persist.tile([128, KT * D], BF16, tag="wp")
            for k in range(KT):
                nc.sync.dma_start(out=wp_sb[:, k * D:(k + 1) * D],
                                  in_=wp[k * 128:(k + 1) * 128, :])
            id_sb = persist.tile([128, 64], BF16, tag="ident")
            make_identity(nc, id_sb[0:64, :])
            make_identity(nc, id_sb[64:128, :])
            ones_sb = persist.tile([1, 64], BF16, tag="ones")
            nc.vector.memset(ones_sb[:], 1.0)

            # exp activation-table preload
            warm_exp = persist.tile([1, 64], BF16, tag="warmexp")
            nc.scalar.activation(warm_exp[:], id_sb[0:1, :], AF.Exp)

            # bias row replicated across partitions via K=1 outer product
            onesf = persist.tile([1, 128], F32, tag="onesf")
            nc.vector.memset(onesf[:], 1.0)
            bp_row = persist.tile([1, D], F32, tag="bprow")
            nc.sync.dma_start(out=bp_row[:], in_=bp[:, :])
            bp_rep = persist.tile([128, D], F32, tag="bprep")
            for h in range(2):
                rp = fill_pool.tile([128, 512], F32, tag="fill", name=f"bpr{h}")
                nc.tensor.matmul(rp[:], lhsT=onesf[:],
                                 rhs=bp_row[:, h * 512:(h + 1) * 512],
                                 start=True, stop=True)
                nc.vector.tensor_copy(bp_rep[:, h * 512:(h + 1) * 512], rp[:])

            # ---------------- QKV machinery ----------------
            # per-batch SBUF tensors (no cross-batch AP aliasing)
            qkv_sb = [[persist.tile([128, N], BF16, tag=f"qkv{b}{m}",
                                    name=f"qkv{b}{m}") for m in range(3)]
                      for b in range(B)]
            vaug = [persist.tile([128, HPC * 16 * 65], BF16, tag=f"vaug{b}",
                                 name=f"vaug{b}") for b in range(B)]
            for b in range(B):
                nc.vector.memset(vaug[b][:], 1.0)

            xt_sb = {}

            def emit_xt(b, nch, eng=None):
                # nch is batch-local (0..3); loads 8 k-tiles of 512 tokens
                eng = eng or nc.sync
                tiles = []
                for k in range(KT):
                    t = xt_pool.tile([128, 512], BF16, tag="xt",
                                     name=f"xt{b}_{nch}_{k}")
                    eng.dma_start(
                        out=t[:],
                        in_=xT[k * 128:(k + 1) * 128,
                               b * N + nch * 512: b * N + (nch + 1) * 512])
                    tiles.append(t)
                xt_sb[(b, nch)] = tiles

            def emit_qkv_half(b, nch, m, half, on_act):
                # one half (4 k-tiles) of a [128,512] qkv chunk accumulation
                key = (b, nch, m)
                if half == 0:
                    acc = fill_pool.tile([128, 512], F32, tag="fill",
                                         name=f"acc{b}_{nch}_{m}")
                    qkv_acc[key] = acc
                acc = qkv_acc[key]
                for k in range(4 * half, 4 * half + 4):
                    nc.tensor.matmul(
                        acc[:],
                        lhsT=w_sb[:, k * 6 * HD + m * 128:
                                  k * 6 * HD + (m + 1) * 128],
                        rhs=xt_sb[(b, nch)][k][:],
                        start=(k == 0), stop=(k == KT - 1))
                if half == 1:
                    nsl = slice(nch * 512, (nch + 1) * 512)
                    if on_act:
                        nc.scalar.copy(qkv_sb[b][m][:, nsl], acc[:])
                    else:
                        nc.vector.tensor_copy(qkv_sb[b][m][:, nsl], acc[:])
                    qkv_acc.pop(key)

            def emit_vt(b, nch):
                # transpose the v chunk into V_aug lhsT tiles
                v_sb = qkv_sb[b][2]
                for jj in range(4):
                    nloc = nch * 512 + jj * 128
                    jt = nloc // 128
                    for p in range(HPC):
                        tp = fill_pool.tile([128, 512], BF16, tag="fill",
                                            name=f"tp{b}_{nch}_{jj}_{p}")
                        nc.tensor.transpose(
                            tp[:, 0:64], v_sb[p * 64:(p + 1) * 64,
                                              nloc:nloc + 128],
                            id_sb[p * 64:(p + 1) * 64, :])
                        base = (p * 16 + jt) * 65
                        nc.vector.tensor_copy(vaug[b][:, base:base + 64],
                                              tp[:, 0:64])

            qkv_acc = {}

            # ---------------- cbt prefetch machinery ----------------
            PF = 8
            cb_tiles = {}

            def emit_cbt(T):
                if T >= 128:
                    return
                Gq, jtq = divmod(T, 16)
                bq, ggq = divmod(Gq, NB)
                t = cb_pool.tile([128, 1024], BF16, tag="cbt", name=f"cbt{T}")
                nc.sync.dma_start(
                    out=t[:], in_=cb2[bq, ggq, jtq * 128:(jtq + 1) * 128, :])
                cb_tiles[T] = t

            # ---------------- phase 0: QKV for batch 0 ----------------
            # order on the sync queue: first 2 b0 x-chunks, then the cbt
            # primes (needed at attention iter 0), then the rest of b0's x.
            # b1's x goes on the scalar HWDGE ring: zero sync contention.
            emit_xt(0, 0)
            emit_xt(0, 1)
            for T in range(PF):
                emit_cbt(T)
            emit_xt(0, 2)
            emit_xt(0, 3)
            for nch in range(4):
                emit_xt(1, nch, eng=nc.scalar)
            for nch in range(4):
                for m in range(3):
                    emit_qkv_half(0, nch, m, 0, True)
                    emit_qkv_half(0, nch, m, 1, True)
                emit_vt(0, nch)

            # batch-1 QKV is interleaved into batch-0 attention: small
            # halves mid-group, dense 8-matmul bursts at group boundaries
            # (a >=3.4us contiguous PE stream re-warms the HAM clock gate)
            qkv_half_seq = [(u // 3, u % 3, h) for u in range(12)
                            for h in (0, 1)]
            half_idx = [0]

            def next_qkv_half():
                if half_idx[0] >= len(qkv_half_seq):
                    return
                nch, m, h = qkv_half_seq[half_idx[0]]
                half_idx[0] += 1
                emit_qkv_half(1, nch, m, h, False)
                if m == 2 and h == 1:
                    emit_vt(1, nch)

            # ---------------- projection machinery ----------------
            og_tiles = {}

            def emit_og(c):
                tiles = []
                for k in range(KT):
                    ogt = og_pool.tile([128, 128], BF16, tag="og",
                                       name=f"og{c}_{k}")
                    nc.gpsimd.dma_start(out=ogt[:], in_=cc_out[c, k])
                    tiles.append(ogt)
                og_tiles[c] = tiles

            def emit_proj_half(c, h):
                # out[t, o-half] for this core's 128-token slice of chunk c
                pps = fill_pool.tile([128, 512], F32, tag="fill",
                                     name=f"pps{c}_{h}")
                for k in range(KT):
                    nc.tensor.matmul(
                        pps[:],
                        lhsT=og_tiles[c][k][:],
                        rhs=wp_sb[:, k * D + h * 512: k * D + (h + 1) * 512],
                        start=(k == 0), stop=(k == KT - 1))
                if h == 1:
                    og_tiles.pop(c)
                outt = out_pool.tile([128, 512], F32, tag="outt",
                                     name=f"outt{c}_{h}")
                nc.vector.tensor_tensor(outt[:], pps[:],
                                        bp_rep[:, h * 512:(h + 1) * 512],
                                        OP.add)
                nc.sync.dma_start(
                    out=out_ext[c * 128:(c + 1) * 128,
                                h * 512:(h + 1) * 512],
                    in_=outt[:])

            # ---------------- attention ----------------
            # oT per 1024-token chunk
            oT_c = [persist.tile([128, 1024], BF16, tag=f"oT{c}",
                                 name=f"oT{c}") for c in range(4)]

            ots_map = {}

            def get_ots(G):
                if G not in ots_map:
                    ots_map[G] = [ot_pool.tile([65, 512], F32, tag="ot",
                                               name=f"ot{G}_{p}")
                                  for p in range(HPC)]
                return ots_map[G]

            def emit_pv(item):
                Gp, jtp, pw = item
                ots = get_ots(Gp)
                for p in range(HPC):
                    base = (p * 16 + jtp) * 65
                    nc.tensor.matmul(
                        ots[p][:],
                        lhsT=vaug[Gp // NB][:, base:base + 65],
                        rhs=pw[:, p * 512:(p + 1) * 512],
                        start=(jtp == 0), stop=(jtp == 15))

            def finalize_group(G):
                # softmax normalize + stage into oT; on odd groups fire the
                # chunk AllToAll. Runs 2 iterations into the NEXT group so
                # the QK/exp stream never pauses at a group boundary.
                b, gg = divmod(G, NB)
                c = (b * N + gg * 512) // 1024
                ots = ots_map.pop(G)
                for p in range(HPC):
                    ot = ots[p]
                    recf = small_pool.tile([1, 512], F32, tag="recf")
                    sums = small_pool.tile([1, 512], F32, tag="sums")
                    nc.vector.tensor_copy(sums[:], ot[64:65, :])
                    nc.vector.reciprocal_approx_fast(recf[:], sums[:])
                    rep = small_pool.tile([64, 512], F32, tag="rep")
                    nc.gpsimd.partition_broadcast(rep[:], recf[:])
                    nc.vector.tensor_tensor(
                        oT_c[c][p * 64:(p + 1) * 64,
                                (G % 2) * 512:(G % 2) * 512 + 512],
                        ot[0:64, :], rep[:], OP.mult)
                if G % 2 == 1:
                    for j in range(NCORES):
                        nc.sync.dma_start(
                            out=cc_in[c, j],
                            in_=oT_c[c][:, j * 128:(j + 1) * 128])
                    nc.gpsimd.collective_compute(
                        "AllToAll", OP.bypass, replica_groups=groups,
                        ins=[cc_in[c].opt()], outs=[cc_out[c].opt()])
                if G == 1:
                    pass
                elif G == 3:
                    emit_og(0)
                elif G == 4:
                    emit_og(1)
                elif G == 5:
                    emit_og(2)   # waits on A2A(2) on the idle SWDGE queue

            pend = []
            for T in range(128):
                G, jt = divmod(T, 16)
                b, gg = divmod(G, NB)
                isl = slice(gg * 512, (gg + 1) * 512)
                emit_cbt(T + PF)
                cbt = cb_tiles.pop(T)
                st = st_pool.tile([128, 1024], F32, tag="st")
                for p in range(HPC):
                    nc.tensor.matmul(
                        st[:, p * 512:(p + 1) * 512],
                        lhsT=qkv_sb[b][1][p * 64:(p + 1) * 64,
                                          jt * 128:(jt + 1) * 128],
                        rhs=qkv_sb[b][0][p * 64:(p + 1) * 64, isl],
                        start=True, stop=True)
                raw = sw_pool.tile([128, 1024], BF16, tag="sw")
                nc.scalar.activation(raw[:], st[:], AF.Exp)
                pw = pw_pool.tile([128, 1024], BF16, tag="pw")
                nc.vector.tensor_tensor(pw[:], raw[:], cbt[:], OP.mult)
                # finalize the previous group before its first PV pops so
                # the ot pool slots are recycled in order
                if jt == 2 and G >= 1:
                    finalize_group(G - 1)
                pend.append((G, jt, pw))
                if len(pend) > 2:
                    emit_pv(pend.pop(0))
                # PE filler, kept away from group boundaries
                if G < NB and jt in (2, 4, 6, 8, 10, 12):
                    next_qkv_half()
                elif G == 5 and jt in (4, 10):
                    emit_proj_half(0, 1 if jt == 10 else 0)
                elif G == 6 and jt in (4, 10):
                    emit_proj_half(1, 1 if jt == 10 else 0)
            while pend:
                emit_pv(pend.pop(0))
            finalize_group(7)

            # tail
            emit_proj_half(2, 0)
            emit_proj_half(2, 1)
            emit_og(3)
            emit_proj_half(3, 0)
            emit_proj_half(3, 1)

    nc.compile()
    return nc


_GRAPH = None


def _get_graph():
    global _GRAPH
    if _GRAPH is None:
        _GRAPH = _build_graph()
    return _GRAPH


def kernel(x, attn_bias, attn_mask, w_qkv, w_proj, b_proj):
    global LAST_EXEC_TIME_NS
    bf16 = ml_dtypes.bfloat16
    x = np.asarray(x, np.float32)
    attn_bias = np.asarray(attn_bias, np.float32)
    attn_mask = np.asarray(attn_mask)
    w_qkv = np.asarray(w_qkv, np.float32)
    w_proj = np.asarray(w_proj, np.float32)
    b_proj = np.asarray(b_proj, np.float32)

    scale = np.float32(HD ** -0.5)
    xT = np.ascontiguousarray(x.reshape(NT, D).T).astype(bf16)
    wq, wk, wv = w_qkv[0:D], w_qkv[D:2 * D], w_qkv[2 * D:3 * D]
    maskvalT = np.where(attn_mask, np.float32(MASK_NEG),
                        np.float32(0.0)).transpose(0, 2, 1)  # [B, j, i]
    biasT = attn_bias[0].transpose(0, 2, 1)                  # [H, j, i]
    wp_np = np.ascontiguousarray(w_proj.T).astype(bf16)      # [D(d), D(o)]
    bp_np = b_proj.reshape(1, D).astype(np.float32)

    in_maps = []
    for cid in range(NCORES):
        hs = [HPC * cid + p for p in range(HPC)]
        wcols = np.concatenate(
            [wq[h * HD:(h + 1) * HD] * scale for h in hs]
            + [wk[h * HD:(h + 1) * HD] for h in hs]
            + [wv[h * HD:(h + 1) * HD] for h in hs], axis=0)   # [384, D]
        wqkvT_np = np.ascontiguousarray(wcols.T).astype(bf16)  # [D, 384]
        cb_np = np.empty((B, NB, N, HPC * 512), dtype=bf16)
        for b in range(B):
            for p, h in enumerate(hs):
                with np.errstate(under="ignore"):
                    plane = np.exp(biasT[h] + maskvalT[b]).astype(bf16)
                for gg in range(NB):
                    cb_np[b, gg, :, p * 512:(p + 1) * 512] = \
                        plane[:, gg * 512:(gg + 1) * 512]
        in_maps.append({"xT": xT, "wqkvT": wqkvT_np, "cb2": cb_np,
                        "wp": wp_np, "bp": bp_np})

    nc = _get_graph()
    trace = bool(os.environ.get("BASS_PROF"))
    res = run_bass_kernel_spmd(nc, in_maps, core_ids=list(range(NCORES)),
                               trace=trace)
    LAST_EXEC_TIME_NS = res.exec_time_ns
    # res[c]["out"]: [4*128, D] = out.T rows for tokens c*... reassemble:
    # global token (chunk ch, core cid, i) = ch*1024 + cid*128 + i
    outT = np.empty((NT, D), np.float32)
    for cid in range(NCORES):
        o = res.results[cid]["out"]
        for ch in range(4):
            outT[ch * 1024 + cid * 128: ch * 1024 + (cid + 1) * 128] = \
                o[ch * 128:(ch + 1) * 128]
    return outT.reshape(B, N, D).astype(np.float32)


# revision 15
# speedup vs baseline: 1.1251x; 1.0900x over previous
"""Distributed multi-head attention kernel for 8 TRN2 NeuronCores.

Head-parallel tensor parallelism: each core owns 2 of the 16 heads.
Compute in bf16 (f32 PSUM accumulation). Scores are computed transposed
(ST[j,i] = k_j . q_i) so that the softmax denominator rides the PV matmul
via a ones-column in V and no transpose of P is needed.
No max-subtraction softmax: logits are O(10), exp stays in f32 range.

v3 structure:
  - Attention runs over 8 groups of 512 query tokens; each (group, jt)
    iteration computes BOTH heads' [128,512] score blocks into one
    [128,1024] PSUM tile so softmax needs ONE exp ACTIVATE and ONE
    bf16 multiply per iteration (ACT is the pace-setting engine).
  - PSUM: 2x st [128,1024] (4 banks) + 2x ot [65,512] (2 banks) +
    2x fill [128,512] (2 banks) = 8 banks.
  - PE filler work (batch-1 QKV, output projection) is interleaved into
    the attention iterations so the tensor engine stream stays dense and
    the HAM clock gate stays at full rate.
  - Token-parallel output projection: per 1024-token chunk, an AllToAll
    exchanges head-blocks for token-blocks (8x less wire than AllGather),
    then each core projects its own 128-token slice against the full
    w_proj, writing out.T ([tokens, D]) directly.
"""

import os
import numpy as np
import ml_dtypes

import concourse.bass as bass
import concourse.mybir as mybir
import concourse.tile as tile
from concourse import bacc
from concourse.bass_utils import run_bass_kernel_spmd
from concourse.masks import make_identity

BF16 = mybir.dt.bfloat16
F32 = mybir.dt.float32
AF = mybir.ActivationFunctionType
OP = mybir.AluOpType

NCORES = 8
B, N, D, H, HD = 2, 2048, 1024, 16, 64
NT = B * N
HPC = H // NCORES     # 2 heads per core
NB = N // 512         # 4 groups of 512 tokens per batch
MASK_NEG = -30000.0

LAST_EXEC_TIME_NS = None


def _build_graph():
    nc = bacc.Bacc("TRN2", target_bir_lowering=False, debug=False, num_devices=NCORES)

    xT = nc.declare_dram_parameter("xT", [D, NT], BF16, isOutput=False)
    wqkvT = nc.declare_dram_parameter("wqkvT", [D, 6 * HD], BF16, isOutput=False)
    # cb2[b, gg, j, p*512+i] = exp(bias[h_p, j, gg*512+i] + maskval[b, j, ...])
    cb2 = nc.declare_dram_parameter("cb2", [B, NB, N, HPC * 512], BF16,
                                    isOutput=False)
    wp = nc.declare_dram_parameter("wp", [D, D], BF16, isOutput=False)
    bp = nc.declare_dram_parameter("bp", [1, D], F32, isOutput=False)
    # out.T: this core's 4x128 tokens (one 128-block per 1024-token chunk)
    out_ext = nc.declare_dram_parameter("out", [4 * 128, D], F32, isOutput=True)

    # AllToAll bounce buffers: per chunk, 8 shards of [128 rows, 128 tokens]
    cc_in = nc.dram_tensor("cc_in", [4, NCORES, 128, 128], BF16)
    cc_out = nc.dram_tensor("cc_out", [4, NCORES, 128, 128], BF16)
    cc_warm_in = nc.dram_tensor("cc_warm_in", [NCORES, 128, 128], BF16)
    cc_warm_out = nc.dram_tensor("cc_warm_out", [NCORES, 128, 128], BF16)
    groups = [list(range(NCORES))]

    KT = D // 128  # 8 contraction tiles

    with tile.TileContext(nc) as tc:
        with (
            tc.tile_pool(name="persist", bufs=1) as persist,
            tc.tile_pool(name="xt", bufs=64) as xt_pool,
            tc.tile_pool(name="st", bufs=2, space="PSUM") as st_pool,
            tc.tile_pool(name="ot_ps", bufs=2, space="PSUM") as ot_pool,
            tc.tile_pool(name="fill", bufs=2, space="PSUM") as fill_pool,
            tc.tile_pool(name="sw", bufs=6) as sw_pool,
            tc.tile_pool(name="pw", bufs=4) as pw_pool,
            tc.tile_pool(name="cbt", bufs=10) as cb_pool,
            tc.tile_pool(name="small", bufs=4) as small_pool,
            tc.tile_pool(name="og", bufs=16) as og_pool,
            tc.tile_pool(name="outt", bufs=2) as out_pool,
        ):
            # ---------------- warmups ----------------
            nc.sync.dma_start(out=cc_warm_in[0], in_=xT[0:128, 0:128])
            nc.gpsimd.collective_compute(
                "AllToAll", OP.bypass, replica_groups=groups,
                ins=[cc_warm_in[:, :, :].opt()], outs=[cc_warm_out[:, :, :].opt()])

            # ---------------- persistent tensors ----------------
            w_sb = persist.tile([128, KT * 6 * HD], BF16, tag="w")
            for k in range(KT):
                nc.sync.dma_start(
                    out=w_sb[:, k * 6 * HD:(k + 1) * 6 * HD],
                    in_=wqkvT[k * 128:(k + 1) * 128, :])
            wp_sb = persist.tile([128, KT * D], BF16, tag="wp")
            for k in range(KT):
                nc.sync.dma_start(out=wp_sb[:, k * D:(k + 1) * D],
                                  in_=wp[k * 128:(k + 1) * 128, :])
            id_sb = persist.tile([128, 64], BF16, tag="ident")
            make_identity(nc, id_sb[0:64, :])
            make_identity(nc, id_sb[64:128, :])
            ones_sb = persist.tile([1, 64], BF16, tag="ones")
            nc.vector.memset(ones_sb[:], 1.0)

            # exp activation-table preload
            warm_exp = persist.tile([1, 64], BF16, tag="warmexp")
            nc.scalar.activation(warm_exp[:], id_sb[0:1, :], AF.Exp)

            # bias row replicated across partitions via K=1 outer product
            onesf = persist.tile([1, 128], F32, tag="onesf")
            nc.vector.memset(onesf[:], 1.0)
            bp_row = persist.tile([1, D], F32, tag="bprow")
            nc.sync.dma_start(out=bp_row[:], in_=bp[:, :])
            bp_rep = persist.tile([128, D], F32, tag="bprep")
            for h in range(2):
                rp = fill_pool.tile([128, 512], F32, tag="fill", name=f"bpr{h}")
                nc.tensor.matmul(rp[:], lhsT=onesf[:],
                                 rhs=bp_row[:, h * 512:(h + 1) * 512],
                                 start=True, stop=True)
                nc.vector.tensor_copy(bp_rep[:, h * 512:(h + 1) * 512], rp[:])

            # ---------------- QKV machinery ----------------
            # per-batch SBUF tensors (no cross-batch AP aliasing)
            qkv_sb = [[persist.tile([128, N], BF16, tag=f"qkv{b}{m}",
                                    name=f"qkv{b}{m}") for m in range(3)]
                      for b in range(B)]
            vaug = [persist.tile([128, HPC * 16 * 65], BF16, tag=f"vaug{b}",
                                 name=f"vaug{b}") for b in range(B)]
            for b in range(B):
                nc.vector.memset(vaug[b][:], 1.0)

            xt_sb = {}

            def emit_xt(b, nch, eng=None):
                # nch is batch-local (0..3); loads 8 k-tiles of 512 tokens
                eng = eng or nc.sync
                tiles = []
                for k in range(KT):
                    t = xt_pool.tile([128, 512], BF16, tag="xt",
                                     name=f"xt{b}_{nch}_{k}")
                    eng.dma_start(
                        out=t[:],
                        in_=xT[k * 128:(k + 1) * 128,
                               b * N + nch * 512: b * N + (nch + 1) * 512])
                    tiles.append(t)
                xt_sb[(b, nch)] = tiles

            def emit_qkv_half(b, nch, m, half, on_act):
                # one half (4 k-tiles) of a [128,512] qkv chunk accumulation
                key = (b, nch, m)
                if half == 0:
                    acc = fill_pool.tile([128, 512], F32, tag="fill",
                                         name=f"acc{b}_{nch}_{m}")
                    qkv_acc[key] = acc
                acc = qkv_acc[key]
                for k in range(4 * half, 4 * half + 4):
                    nc.tensor.matmul(
                        acc[:],
                        lhsT=w_sb[:, k * 6 * HD + m * 128:
                                  k * 6 * HD + (m + 1) * 128],
                        rhs=xt_sb[(b, nch)][k][:],
                        start=(k == 0), stop=(k == KT - 1))
                if half == 1:
                    nsl = slice(nch * 512, (nch + 1) * 512)
                    if on_act:
                        nc.scalar.copy(qkv_sb[b][m][:, nsl], acc[:])
                    else:
                        nc.vector.tensor_copy(qkv_sb[b][m][:, nsl], acc[:])
                    qkv_acc.pop(key)

            def emit_vt(b, nch):
                # transpose the v chunk into V_aug lhsT tiles
                v_sb = qkv_sb[b][2]
                for jj in range(4):
                    nloc = nch * 512 + jj * 128
                    jt = nloc // 128
                    for p in range(HPC):
                        tp = fill_pool.tile([128, 512], BF16, tag="fill",
                                            name=f"tp{b}_{nch}_{jj}_{p}")
                        nc.tensor.transpose(
                            tp[:, 0:64], v_sb[p * 64:(p + 1) * 64,
                                              nloc:nloc + 128],
                            id_sb[p * 64:(p + 1) * 64, :])
                        base = (p * 16 + jt) * 65
                        nc.vector.tensor_copy(vaug[b][:, base:base + 64],
                                              tp[:, 0:64])

            qkv_acc = {}

            # ---------------- cbt prefetch machinery ----------------
            PF = 8
            cb_tiles = {}

            def emit_cbt(T):
                if T >= 128:
                    return
                Gq, jtq = divmod(T, 16)
                bq, ggq = divmod(Gq, NB)
                t = cb_pool.tile([128, 1024], BF16, tag="cbt", name=f"cbt{T}")
                nc.sync.dma_start(
                    out=t[:], in_=cb2[bq, ggq, jtq * 128:(jtq + 1) * 128, :])
                cb_tiles[T] = t

            # ---------------- phase 0: QKV for batch 0 ----------------
            # order on the sync queue: first 2 b0 x-chunks, then the cbt
            # primes (needed at attention iter 0), then the rest of b0's x.
            # b1's x goes on the scalar HWDGE ring: zero sync contention.
            for T in range(PF):
                emit_cbt(T)
            for bb in range(B):
                for nch in range(4):
                    emit_xt(bb, nch)
            for nch in range(4):
                for m in range(3):
                    emit_qkv_half(0, nch, m, 0, True)
                    emit_qkv_half(0, nch, m, 1, True)
                emit_vt(0, nch)

            # batch-1 QKV is interleaved into batch-0 attention: small
            # halves mid-group, dense 8-matmul bursts at group boundaries
            # (a >=3.4us contiguous PE stream re-warms the HAM clock gate)
            qkv_half_seq = [(u // 3, u % 3, h) for u in range(12)
                            for h in (0, 1)]
            half_idx = [0]

            def next_qkv_half():
                if half_idx[0] >= len(qkv_half_seq):
                    return
                nch, m, h = qkv_half_seq[half_idx[0]]
                half_idx[0] += 1
                emit_qkv_half(1, nch, m, h, False)
                if m == 2 and h == 1:
                    emit_vt(1, nch)

            # ---------------- projection machinery ----------------
            og_tiles = {}

            def emit_og(c):
                tiles = []
                for k in range(KT):
                    ogt = og_pool.tile([128, 128], BF16, tag="og",
                                       name=f"og{c}_{k}")
                    nc.gpsimd.dma_start(out=ogt[:], in_=cc_out[c, k])
                    tiles.append(ogt)
                og_tiles[c] = tiles

            def emit_proj_half(c, h):
                # out[t, o-half] for this core's 128-token slice of chunk c
                pps = fill_pool.tile([128, 512], F32, tag="fill",
                                     name=f"pps{c}_{h}")
                for k in range(KT):
                    nc.tensor.matmul(
                        pps[:],
                        lhsT=og_tiles[c][k][:],
                        rhs=wp_sb[:, k * D + h * 512: k * D + (h + 1) * 512],
                        start=(k == 0), stop=(k == KT - 1))
                if h == 1:
                    og_tiles.pop(c)
                outt = out_pool.tile([128, 512], F32, tag="outt",
                                     name=f"outt{c}_{h}")
                nc.vector.tensor_tensor(outt[:], pps[:],
                                        bp_rep[:, h * 512:(h + 1) * 512],
                                        OP.add)
                nc.scalar.dma_start(
                    out=out_ext[c * 128:(c + 1) * 128,
                                h * 512:(h + 1) * 512],
                    in_=outt[:])

            # ---------------- attention ----------------
            # oT per 1024-token chunk
            oT_c = [persist.tile([128, 1024], BF16, tag=f"oT{c}",
                                 name=f"oT{c}") for c in range(4)]

            ots_map = {}

            def get_ots(G):
                if G not in ots_map:
                    ots_map[G] = [ot_pool.tile([65, 512], F32, tag="ot",
                                               name=f"ot{G}_{p}")
                                  for p in range(HPC)]
                return ots_map[G]

            def emit_pv(item):
                Gp, jtp, pw = item
                ots = get_ots(Gp)
                for p in range(HPC):
                    base = (p * 16 + jtp) * 65
                    nc.tensor.matmul(
                        ots[p][:],
                        lhsT=vaug[Gp // NB][:, base:base + 65],
                        rhs=pw[:, p * 512:(p + 1) * 512],
                        start=(jtp == 0), stop=(jtp == 15))

            def finalize_group(G):
                # softmax normalize + stage into oT; on odd groups fire the
                # chunk AllToAll. Runs 2 iterations into the NEXT group so
                # the QK/exp stream never pauses at a group boundary.
                b, gg = divmod(G, NB)
                c = (b * N + gg * 512) // 1024
                ots = ots_map.pop(G)
                for p in range(HPC):
                    ot = ots[p]
                    recf = small_pool.tile([1, 512], F32, tag="recf")
                    sums = small_pool.tile([1, 512], F32, tag="sums")
                    nc.vector.tensor_copy(sums[:], ot[64:65, :])
                    nc.vector.reciprocal_approx_fast(recf[:], sums[:])
                    rep = small_pool.tile([64, 512], F32, tag="rep")
                    nc.gpsimd.partition_broadcast(rep[:], recf[:])
                    nc.vector.tensor_tensor(
                        oT_c[c][p * 64:(p + 1) * 64,
                                (G % 2) * 512:(G % 2) * 512 + 512],
                        ot[0:64, :], rep[:], OP.mult)
                if G % 2 == 1:
                    for j in range(NCORES):
                        nc.gpsimd.dma_start(
                            out=cc_in[c, j],
                            in_=oT_c[c][:, j * 128:(j + 1) * 128])
                    nc.gpsimd.collective_compute(
                        "AllToAll", OP.bypass, replica_groups=groups,
                        ins=[cc_in[c].opt()], outs=[cc_out[c].opt()])
                if G == 1:
                    pass
                elif G == 3:
                    emit_og(0)
                elif G == 4:
                    emit_og(1)
                elif G == 5:
                    emit_og(2)   # waits on A2A(2) on the idle SWDGE queue

            pend = []
            for T in range(128):
                G, jt = divmod(T, 16)
                b, gg = divmod(G, NB)
                isl = slice(gg * 512, (gg + 1) * 512)
                emit_cbt(T + PF)
                cbt = cb_tiles.pop(T)
                st = st_pool.tile([128, 1024], F32, tag="st")
                for p in range(HPC):
                    nc.tensor.matmul(
                        st[:, p * 512:(p + 1) * 512],
                        lhsT=qkv_sb[b][1][p * 64:(p + 1) * 64,
                                          jt * 128:(jt + 1) * 128],
                        rhs=qkv_sb[b][0][p * 64:(p + 1) * 64, isl],
                        start=True, stop=True)
                raw = sw_pool.tile([128, 1024], BF16, tag="sw")
                nc.scalar.activation(raw[:], st[:], AF.Exp)
                pw = pw_pool.tile([128, 1024], BF16, tag="pw")
                nc.vector.tensor_tensor(pw[:], raw[:], cbt[:], OP.mult)
                # finalize the previous group before its first PV pops so
                # the ot pool slots are recycled in order
                if jt == 2 and G >= 1:
                    finalize_group(G - 1)
                pend.append((G, jt, pw))
                if len(pend) > 2:
                    emit_pv(pend.pop(0))
                # PE filler, kept away from group boundaries
                if G < NB and jt in (2, 4, 6, 8, 10, 12):
                    next_qkv_half()
                elif G == 5 and jt in (4, 10):
                    emit_proj_half(0, 1 if jt == 10 else 0)
                elif G == 6 and jt in (4, 10):
                    emit_proj_half(1, 1 if jt == 10 else 0)
            while pend:
                emit_pv(pend.pop(0))
            finalize_group(7)

            # tail
            emit_proj_half(2, 0)
            emit_proj_half(2, 1)
            emit_og(3)
            emit_proj_half(3, 0)
            emit_proj_half(3, 1)

    nc.compile()
    return nc


_GRAPH = None


def _get_graph():
    global _GRAPH
    if _GRAPH is None:
        _GRAPH = _build_graph()
    return _GRAPH


def kernel(x, attn_bias, attn_mask, w_qkv, w_proj, b_proj):
    global LAST_EXEC_TIME_NS
    bf16 = ml_dtypes.bfloat16
    x = np.asarray(x, np.float32)
    attn_bias = np.asarray(attn_bias, np.float32)
    attn_mask = np.asarray(attn_mask)
    w_qkv = np.asarray(w_qkv, np.float32)
    w_proj = np.asarray(w_proj, np.float32)
    b_proj = np.asarray(b_proj, np.float32)

    scale = np.float32(HD ** -0.5)
    xT = np.ascontiguousarray(x.reshape(NT, D).T).astype(bf16)
    wq, wk, wv = w_qkv[0:D], w_qkv[D:2 * D], w_qkv[2 * D:3 * D]
    maskvalT = np.where(attn_mask, np.float32(MASK_NEG),
                        np.float32(0.0)).transpose(0, 2, 1)  # [B, j, i]
    biasT = attn_bias[0].transpose(0, 2, 1)                  # [H, j, i]
    wp_np = np.ascontiguousarray(w_proj.T).astype(bf16)      # [D(d), D(o)]
    bp_np = b_proj.reshape(1, D).astype(np.float32)

    in_maps = []
    for cid in range(NCORES):
        hs = [HPC * cid + p for p in range(HPC)]
        wcols = np.concatenate(
            [wq[h * HD:(h + 1) * HD] * scale for h in hs]
            + [wk[h * HD:(h + 1) * HD] for h in hs]
            + [wv[h * HD:(h + 1) * HD] for h in hs], axis=0)   # [384, D]
        wqkvT_np = np.ascontiguousarray(wcols.T).astype(bf16)  # [D, 384]
        cb_np = np.empty((B, NB, N, HPC * 512), dtype=bf16)
        for b in range(B):
            for p, h in enumerate(hs):
                with np.errstate(under="ignore"):
                    plane = np.exp(biasT[h] + maskvalT[b]).astype(bf16)
                for gg in range(NB):
                    cb_np[b, gg, :, p * 512:(p + 1) * 512] = \
                        plane[:, gg * 512:(gg + 1) * 512]
        in_maps.append({"xT": xT, "wqkvT": wqkvT_np, "cb2": cb_np,
                        "wp": wp_np, "bp": bp_np})

    nc = _get_graph()
    trace = bool(os.environ.get("BASS_PROF"))
    res = run_bass_kernel_spmd(nc, in_maps, core_ids=list(range(NCORES)),
                               trace=trace)
    LAST_EXEC_TIME_NS = res.exec_time_ns
    # res[c]["out"]: [4*128, D] = out.T rows for tokens c*... reassemble:
    # global token (chunk ch, core cid, i) = ch*1024 + cid*128 + i
    outT = np.empty((NT, D), np.float32)
    for cid in range(NCORES):
        o = res.results[cid]["out"]
        for ch in range(4):
            outT[ch * 1024 + cid * 128: ch * 1024 + (cid + 1) * 128] = \
                o[ch * 128:(ch + 1) * 128]
    return outT.reshape(B, N, D).astype(np.float32)
